# revision 1
# baseline (speedup 1.0000x reference)
"""Multi-head causal attention (B=4, S=2048, D=1024, 16 heads) on 8 TRN2 cores.

Sharding: core c -> (batch b = c//2, head-group g = c%2). Each core computes
8 heads of one batch element end-to-end (QKV proj, causal softmax attention,
out-proj rows for its head slice). Host sums the two head-group partials per
batch and adds the output bias.

Per-core pipeline (all matmuls contraction-on-partitions, bf16 in / f32 psum):
  QT/KT[dtile] = (x @ w)^T   [128p = 2 heads x 64, S]
  Vones[kb]    = [V | 1]     [128p = k, h, 65]
  attention per (512-wide q-chunk, head-pair); the pair's score matmuls are
  packed into PE row groups via tile_position (concurrent on real HW); two
  k-blocks share one [128,1024] score psum so each exp covers ~1024 cols;
  PV matmuls trail two iterations behind so PE never waits on the exp:
    ST[k,q] = KT.T @ QT; PT = exp(ST/8) bf16; tri-mask on diagonal 128 cols
    ctx[65, 512] += [V|1].T @ PT   (row 64 = softmax denominators)
    cxt = ctx[0:64] * gpsimd-broadcast(1/ctx[64])
  out[seq128, 512] = cxt.T @ ow, streamed to DRAM per q-chunk; the final
  q-chunk's out-proj borrows the freed score psum banks.
"""

import numpy as np
import ml_dtypes

B, S, D = 4, 2048, 1024
H_TOT = 16
HD = 64
NCORES = 8
GH = 8          # heads per core
GD = GH * HD    # 512: dout slice per core
NKB = S // 128  # 16 k-blocks
NQC = S // 512  # 4 q-chunks
BF16 = ml_dtypes.bfloat16

PACK_HEADS = True   # pack 2 heads' score matmuls into PE row groups

_cache = {}


def _build_body(tc, nc, mybir, xT, wq, wk, wv, ow, outp):
    from concourse.masks import make_upper_triangular
    import contextlib

    dt = mybir.dt
    F = mybir.ActivationFunctionType

    pools = contextlib.ExitStack()
    tc_pool = lambda **kw: pools.enter_context(tc.tile_pool(**kw))

    singles = tc_pool(name="singles", bufs=1)
    pt_pool = tc_pool(name="pt", bufs=8)
    small = tc_pool(name="small", bufs=4)
    rb_pool = tc_pool(name="rb", bufs=6)
    ost_pool = tc_pool(name="ost", bufs=5)
    psum_st = tc_pool(name="psum_st", bufs=2, space="PSUM")
    psum_ctx = tc_pool(name="psum_ctx", bufs=3, space="PSUM")
    psum_mm = tc_pool(name="psum_mm", bufs=1, space="PSUM")

    # ---- persistent SBUF tensors (split per producer/consumer region so the
    # dependency tracker never over-serializes) ----
    xT_sb = [singles.tile([128, S], dt.bfloat16, name=f"xt{t}")
             for t in range(8)]
    wq_sb = [singles.tile([128, GD], dt.bfloat16, name=f"wq{t}")
             for t in range(8)]
    wk_sb = [singles.tile([128, GD], dt.bfloat16, name=f"wk{t}")
             for t in range(8)]
    wv_sb = [singles.tile([128, GD], dt.bfloat16, name=f"wv{t}")
             for t in range(8)]
    ow_sb = [singles.tile([128, D], dt.bfloat16, name=f"ow{t}")
             for t in range(4)]
    qt_sb = [singles.tile([128, S], dt.bfloat16, name=f"qt{t}")
             for t in range(4)]                              # 2 heads / dtile
    kt_sb = [singles.tile([128, S], dt.bfloat16, name=f"kt{t}")
             for t in range(4)]
    vo_sb = [singles.tile([128, GH, 65], dt.bfloat16, name=f"vo{t}")
             for t in range(NKB)]                            # [V_h | ones]
    cxt_sb = [singles.tile([128, S], dt.bfloat16, name=f"cxt{t}")
              for t in range(4)]                             # ctx^T normalized
    tri = singles.tile([128, 128], dt.bfloat16)              # keep k<=q

    make_upper_triangular(nc, tri, val=1.0, diag=True)
    for t in range(NKB):
        nc.vector.memset(vo_sb[t][:, :, 64:65], 1.0)

    # ---- input DMAs (split per 128-row tile; first matmuls need wq t0 + xT t0) ----
    xT_r = xT.ap().rearrange("(t p) s -> p t s", p=128)
    wq_r = wq.ap().rearrange("(t p) n -> p t n", p=128)
    wk_r = wk.ap().rearrange("(t p) n -> p t n", p=128)
    wv_r = wv.ap().rearrange("(t p) n -> p t n", p=128)
    ow_r = ow.ap().rearrange("(t p) n -> p t n", p=128)
    # SP queue: even xT tiles then wk; ACT queue: wq (small) then odd xT;
    # gpsimd SWDGE: wv/ow (needed late). Two HWDGE queues halve the
    # serial input-load latency the first projections wait on.
    DIN_ORDER = list(range(8))
    for t in range(8):
        nc.sync.dma_start(out=xT_sb[t], in_=xT_r[:, t, :])
        nc.sync.dma_start(out=wq_sb[t], in_=wq_r[:, t, :])
        if t % 2 == 1:
            nc.sync.dma_start(out=wk_sb[t // 2], in_=wk_r[:, t // 2, :])
    for t in range(4, 8):
        nc.sync.dma_start(out=wk_sb[t], in_=wk_r[:, t, :])
    for t in range(8):
        nc.sync.dma_start(out=wv_sb[t], in_=wv_r[:, t, :])
    for t in range(4):
        nc.sync.dma_start(out=ow_sb[t], in_=ow_r[:, t, :])

    def emit_proj_dtile(w_sb, t_sb, dtile):
        # din-outer: tolerate in-flight xT DMAs; 2 stp slots = 4 psum halves
        pst = [psum_st.tile([128, 1024], dt.float32, name="stp")
               for _ in range(2)]
        pss = [pst[0][:, 0:512], pst[0][:, 512:1024],
               pst[1][:, 0:512], pst[1][:, 512:1024]]
        for i, din in enumerate(DIN_ORDER):
            for c in range(4):
                nc.tensor.matmul(
                    pss[c],
                    lhsT=w_sb[din][:, dtile * 128:(dtile + 1) * 128],
                    rhs=xT_sb[din][:, c * 512:(c + 1) * 512],
                    start=(i == 0),
                    stop=(i == 7),
                )
        for c in range(4):
            nc.vector.tensor_copy(
                out=t_sb[dtile][:, c * 512:(c + 1) * 512], in_=pss[c])

    def emit_v(st):
        ps = psum_mm.tile([128, 512], dt.float32, name="mmps")
        for din in range(8):
            nc.tensor.matmul(
                ps,
                lhsT=xT_sb[din][:, st * 128:(st + 1) * 128],
                rhs=wv_sb[din],
                start=(din == 0),
                stop=(din == 7),
            )
        nc.vector.tensor_copy(
            out=vo_sb[st][:, :, 0:64],
            in_=ps.rearrange("p (h d) -> p h d", h=GH),
        )

    def emit_norm(ctx_tile, h, q0):
        """cxt[h rows, q0:q0+512] = ctx[0:64] * broadcast(1/ctx[64])."""
        recip = small.tile([1, 512], dt.float32, name="recip")
        nc.vector.reciprocal(out=recip, in_=ctx_tile[64:65, :])
        rb = rb_pool.tile([64, 512], dt.float32, name="rb")
        nc.gpsimd.partition_broadcast(rb, recip)
        nc.vector.tensor_mul(
            cxt_sb[h // 2][(h % 2) * 64:(h % 2) * 64 + 64, q0:q0 + 512],
            ctx_tile[0:64, :],
            rb,
        )

    def emit_attn_pair(qc, hp):
        """512-wide q chunk qc for heads h0=2*hp (array rows 0:64) and
        h1=2*hp+1 (rows 64:128); scores packed into PE row groups.

        Two k-blocks share one [128,1024] ST psum tile so each exp covers up
        to 1024 columns. PV matmuls trail by one pair-iteration so the PE
        never sits directly behind the exp on the ACT engine."""
        nkb = 4 * qc + 4
        q0 = 512 * qc
        ctxs = [psum_ctx.tile([65, 512], dt.float32, name="ctx")
                for _ in range(2)]
        pend2 = []

        def emit_pv(kbs, offs, ns, pts):
            for half in range(2):
                for (kb, off, n) in zip(kbs, offs, ns):
                    nc.tensor.matmul(
                        ctxs[half][:, 512 - n:512],
                        lhsT=vo_sb[kb][:, 2 * hp + half, :],
                        rhs=pts[half][:, off:off + n],
                        start=(kb == 0),
                        stop=(kb == nkb - 1),
                    )

        for kb0 in range(0, nkb, 2):
            kbs = [kb for kb in (kb0, kb0 + 1) if kb < nkb]
            ns = [512 - max(0, kb * 128 - q0) for kb in kbs]
            offs = [0] + [ns[0]] * (len(kbs) - 1)
            pts = []
            for half in range(2):
                p0 = half * 64
                stp = psum_st.tile([128, 1024], dt.float32, name="stp")
                for kb, off, n in zip(kbs, offs, ns):
                    nc.tensor.matmul(
                        stp[:, off:off + n],
                        lhsT=kt_sb[hp][p0:p0 + 64, kb * 128:(kb + 1) * 128],
                        rhs=qt_sb[hp][p0:p0 + 64, q0 + 512 - n:q0 + 512],
                        start=True,
                        stop=True,
                        tile_position=(p0, 0) if PACK_HEADS else None,
                    )
                ntot = offs[-1] + ns[-1]
                pt = pt_pool.tile([128, 1024], dt.bfloat16, name="pt")
                nc.scalar.activation(
                    out=pt[:, :ntot], in_=stp[:, :ntot], func=F.Exp,
                    scale=0.125)
                for kb, off in zip(kbs, offs):
                    if kb >= 4 * qc:  # diagonal: mask first 128 cols
                        nc.vector.tensor_mul(
                            pt[:, off:off + 128], pt[:, off:off + 128], tri)
                pts.append(pt)
            pend2.append((kbs, offs, ns, pts))
            if len(pend2) > 2:
                emit_pv(*pend2.pop(0))
        for p in pend2:
            emit_pv(*p)
        for half in range(2):
            emit_norm(ctxs[half], 2 * hp + half, q0)

    def emit_p4(sq_lo, sq_hi, final=False):
        for sq in range(sq_lo, sq_hi):
            for oc in range(2):
                if final:  # attention done: rotate over ALL freed banks
                    k = (sq * 2 + oc) % 3
                    if k == 0:
                        ps = psum_st.tile([128, 1024], dt.float32,
                                          name="stp")[:, 0:512]
                    elif k == 1:
                        ps = psum_ctx.tile([128, 512], dt.float32, name="ctx")
                    else:
                        ps = psum_mm.tile([128, 512], dt.float32, name="mmps")
                else:
                    ps = psum_mm.tile([128, 512], dt.float32, name="mmps")
                for dvt in range(4):
                    nc.tensor.matmul(
                        ps,
                        lhsT=cxt_sb[dvt][:, sq * 128:(sq + 1) * 128],
                        rhs=ow_sb[dvt][:, oc * 512:(oc + 1) * 512],
                        start=(dvt == 0),
                        stop=(dvt == 3),
                    )
                ost = ost_pool.tile([128, 512], dt.float32, name="ost")
                nc.vector.tensor_copy(out=ost, in_=ps)
                nc.sync.dma_start(
                    out=outp.ap()[sq * 128:(sq + 1) * 128,
                                  oc * 512:(oc + 1) * 512],
                    in_=ost,
                )

    # ---- emission schedule: pipeline projections with qc=0 attention ----
    v_ranges = [range(0, 4), range(4, 8), range(8, 12), range(12, 16)]
    for dtile in range(4):
        emit_proj_dtile(wq_sb, qt_sb, dtile)
        emit_proj_dtile(wk_sb, kt_sb, dtile)
        for st in v_ranges[dtile]:
            emit_v(st)
        emit_attn_pair(0, dtile)
    for qc in range(1, NQC):
        for hp in range(4):
            emit_attn_pair(qc, hp)
            if hp == 0:
                emit_p4(4 * (qc - 1), 4 * qc)
    emit_p4(12, 16, final=True)

    return pools


def _build_nc():
    import concourse.tile as tile
    from concourse import bacc, mybir

    dt = mybir.dt
    nc = bacc.Bacc("TRN2", target_bir_lowering=False, debug=False,
                   num_devices=NCORES)
    xT = nc.dram_tensor("xt", [D, S], dt.bfloat16, kind="ExternalInput")
    wq = nc.dram_tensor("wq", [D, GD], dt.bfloat16, kind="ExternalInput")
    wk = nc.dram_tensor("wk", [D, GD], dt.bfloat16, kind="ExternalInput")
    wv = nc.dram_tensor("wv", [D, GD], dt.bfloat16, kind="ExternalInput")
    ow = nc.dram_tensor("ow", [GD, D], dt.bfloat16, kind="ExternalInput")
    outp = nc.dram_tensor("outp", [S, D], dt.float32, kind="ExternalOutput")

    with tile.TileContext(nc) as tc:
        pools = _build_body(tc, nc, mybir, xT, wq, wk, wv, ow, outp)
        pools.close()
    nc.compile()
    return nc


LAST_RESULTS = None


def kernel(batch, w_query, w_key, w_value, out_w, out_b):
    global LAST_RESULTS
    import os
    from concourse import bass_utils

    try:  # BASS_TRACE needs the axon NTFF hook; without it the run crashes
        from antenv.axon_hooks import get_axon_ntff_profile_hook  # noqa: F401
    except ImportError:
        os.environ.setdefault("BASS_NEVER_TRACE", "1")

    batch = np.asarray(batch, dtype=np.float32)
    w_query = np.asarray(w_query, dtype=np.float32)
    w_key = np.asarray(w_key, dtype=np.float32)
    w_value = np.asarray(w_value, dtype=np.float32)
    out_w = np.asarray(out_w, dtype=np.float32)
    out_b = np.asarray(out_b, dtype=np.float32)

    if "nc" not in _cache:
        _cache["nc"] = _build_nc()
    nc = _cache["nc"]

    xts = [np.ascontiguousarray(batch[b].T).astype(BF16) for b in range(B)]
    slc = [slice(g * GD, (g + 1) * GD) for g in range(2)]
    wqs = [np.ascontiguousarray(w_query[:, s]).astype(BF16) for s in slc]
    wks = [np.ascontiguousarray(w_key[:, s]).astype(BF16) for s in slc]
    wvs = [np.ascontiguousarray(w_value[:, s]).astype(BF16) for s in slc]
    ows = [np.ascontiguousarray(out_w[s, :]).astype(BF16) for s in slc]
    in_maps = []
    for c in range(NCORES):
        b, g = divmod(c, 2)
        in_maps.append({
            "xt": xts[b], "wq": wqs[g], "wk": wks[g],
            "wv": wvs[g], "ow": ows[g],
        })

    res = bass_utils.run_bass_kernel_spmd(
        nc, in_maps, core_ids=list(range(NCORES)),
    )
    LAST_RESULTS = res

    out = np.empty((B, S, D), np.float32)
    for b in range(B):
        out[b] = res.results[2 * b]["outp"] + res.results[2 * b + 1]["outp"] \
            + out_b[None, :]
    return out



# revision 29
# speedup vs baseline: 1.1361x; 1.1361x over previous
"""Multi-head causal attention (B=4, S=2048, D=1024, 16 heads) on 8 TRN2 cores.

Sharding: core c -> (batch b = c//2, head-group g = c%2). Each core computes
8 heads of one batch element end-to-end (QKV proj, causal softmax attention,
out-proj rows for its head slice). Host sums the two head-group partials per
batch and adds the output bias.

v3 (bf16 core, flipped PV):
  QT/KT[t] = (x @ w)^T per head-pair t; V into vo rows [V_h0|1|V_h1|1|...].
  scores ST[k, q-span] per pair of k-blocks into one [128,1024] psum; one
  exp per pair on ACT; causal tri-masks on the diagonal sub-blocks (DVE).
  PV is FLIPPED: ctxT[q, 65] = sum_kb (P_kb block)^T [V_kb|1] with lhsT=P,
  so each matmul streams only 65 columns (4.3x fewer PE cycles than the
  [d, q] orientation) and the softmax denominator lands in ctxT col 64.
  Fully-masked (q-block, k-block) combinations are skipped outright.
  Normalization is gpsimd normalize_recip (out = ctxT[:, :64]/ctxT[:, 64]),
  written per q-block into ctn[128, 512]; a DMA xbar transpose turns that
  into cxt[128 d-part, 4 d-tile, q] for the bf16 out-projection.
  Projections for q-chunk qc+1 are interleaved into qc's attention as PE
  filler micro-ops; all out-projs are deferred into the ACT-heavy final
  chunk. The final out-proj copies run on the then-idle ACT engine.
"""

import numpy as np
import ml_dtypes

B, S, D = 4, 2048, 1024
H_TOT = 16
HD = 64
NCORES = 8
GH = 8          # heads per core
GD = GH * HD    # 512: dout slice per core
NKB = S // 128  # 16 k-blocks
NQC = S // 512  # 4 q-chunks
BF16 = ml_dtypes.bfloat16

_cache = {}


def _build_body(tc, nc, mybir, xt_d, wq_d, wk_d, wv_d, ow_d, outp, dbg=None):
    from concourse.masks import make_upper_triangular
    import contextlib

    dt = mybir.dt
    F = mybir.ActivationFunctionType
    bf = dt.bfloat16

    pools = contextlib.ExitStack()
    tc_pool = lambda **kw: pools.enter_context(tc.tile_pool(**kw))

    singles = tc_pool(name="singles", bufs=1)
    pt_pool = tc_pool(name="pt", bufs=6)
    ctf_pool = tc_pool(name="ctf", bufs=3)
    rc_pool = tc_pool(name="rc", bufs=4)
    ctn_pool = tc_pool(name="ctn", bufs=8)
    ost_pool = tc_pool(name="ost", bufs=4)
    psum_st = tc_pool(name="psum_st", bufs=2, space="PSUM")
    psum_ctx = tc_pool(name="psum_ctx", bufs=2, space="PSUM")
    psum_mm = tc_pool(name="psum_mm", bufs=2, space="PSUM")

    # ---- persistent SBUF tensors ----
    xt = singles.tile([128, 8, S], bf, name="xt")
    wq = singles.tile([128, 8, GD], bf, name="wq")
    wk = singles.tile([128, 8, GD], bf, name="wk")
    wv = singles.tile([128, 8, GD], bf, name="wv")
    ow = singles.tile([128, 4, D], bf, name="ow")
    qt = [singles.tile([128, S], bf, name=f"qt{t}") for t in range(4)]
    kt = [singles.tile([128, S], bf, name=f"kt{t}") for t in range(4)]
    VW = GH * 65    # [V_h|1] per head
    vo = singles.tile([128, NKB, VW], bf, name="vo")
    # cxt: (p, t, q) = ctx_norm[t*128+p, q]  (filled by DMA transposes)
    cxt = singles.tile([128, 4, S], bf, name="cxt")
    tri = singles.tile([128, 128], bf)   # keep k<=q
    zero128 = singles.tile([128, 128], bf, name="z128")

    make_upper_triangular(nc, tri, val=1.0, diag=True)
    nc.vector.memset(zero128, 0.0)
    nc.vector.memset(
        vo.rearrange("p k (h e) -> p k h e", e=65)[:, :, :, 64:65], 1.0)

    # ---- input DMAs ----------------------------------------------------
    xt_r = xt_d.ap().rearrange("(t p) s -> p t s", p=128)
    wq_r = wq_d.ap().rearrange("(t p) n -> p t n", p=128)
    wk_r = wk_d.ap().rearrange("(t p) n -> p t n", p=128)
    wv_r = wv_d.ap().rearrange("(t p) n -> p t n", p=128)
    ow_r = ow_d.ap().rearrange("(t p) n -> p t n", p=128)
    for t in range(4):
        nc.scalar.dma_start(out=wk[:, :, t * 128:(t + 1) * 128],
                            in_=wk_r[:, :, t * 128:(t + 1) * 128])
        nc.scalar.dma_start(out=wq[:, :, t * 128:(t + 1) * 128],
                            in_=wq_r[:, :, t * 128:(t + 1) * 128])
    nc.sync.dma_start(out=xt[:, :, 0:512], in_=xt_r[:, :, 0:512])
    nc.sync.dma_start(out=wv, in_=wv_r)
    nc.sync.dma_start(out=xt[:, :, 512:S], in_=xt_r[:, :, 512:S])
    nc.sync.dma_start(out=ow, in_=ow_r)

    # ---- PE work units (lists of micro-ops for fine-grain filling) -----
    def _proj_micros(lhsT_of, rhs_of, copy_fn):
        cell = {}

        def mm(c0):
            if c0 == 0:
                cell["ps"] = psum_mm.tile([128, 512], dt.float32, name="mmps")
            for c in range(c0, c0 + 2):
                nc.tensor.matmul(cell["ps"], lhsT=lhsT_of(c), rhs=rhs_of(c),
                                 start=(c == 0), stop=(c == 7))
            if c0 == 6:
                copy_fn(cell["ps"])
        return [(lambda c0=c0: mm(c0)) for c0 in range(0, 8, 2)]

    def proj_qk(w_sb, t_sb, t, qc):
        q0 = 512 * qc
        return _proj_micros(
            lambda c: w_sb[:, c, t * 128:(t + 1) * 128],
            lambda c: xt[:, c, q0:q0 + 512],
            lambda ps: nc.vector.tensor_copy(out=t_sb[t][:, q0:q0 + 512],
                                             in_=ps))

    def proj_v(kb):
        return _proj_micros(
            lambda c: xt[:, c, kb * 128:(kb + 1) * 128],
            lambda c: wv[:, c, :],
            lambda ps: nc.vector.tensor_copy(
                out=vo[:, kb, :].rearrange("p (h e) -> p h e", e=65)[:, :, 0:64],
                in_=ps.rearrange("p (h e) -> p h e", e=64)))

    def out_proj(sq, tail=False):
        """out rows [128sq,128sq+128) x all 1024 cols; one [128,1024] store."""
        cell = {}

        def oc_unit(oc):
            ps = psum_mm.tile([128, 512], dt.float32, name="mmps")
            for c in range(4):
                nc.tensor.matmul(
                    ps, lhsT=cxt[:, c, sq * 128:(sq + 1) * 128],
                    rhs=ow[:, c, oc * 512:(oc + 1) * 512],
                    start=(c == 0), stop=(c == 3))
            if oc == 0:
                cell["ost"] = ost_pool.tile([128, 1024], bf, name="ost")
            if tail and oc == 0:
                nc.scalar.copy(out=cell["ost"][:, 0:512], in_=ps)
            else:
                nc.vector.tensor_copy(out=cell["ost"][:, oc * 512:(oc + 1) * 512],
                                      in_=ps)
            if oc == 1:
                nc.sync.dma_start(
                    out=outp.ap()[sq * 128:(sq + 1) * 128, :], in_=cell["ost"])
        return [(lambda oc=oc: oc_unit(oc)) for oc in range(2)]

    ctn = {}  # (qc, j) -> staging tile [128 q, 512 d] bf16

    def emit_attn_head(qc, h, fillers):
        """scores+exp+mask for head h / chunk qc; flipped PV trails 2 pairs."""
        t, p0 = h // 2, (h % 2) * 64
        q0 = 512 * qc
        npairs = 2 * qc + 2
        ctxT = psum_ctx.tile([128, 4, 65], dt.float32, name="ctxT")
        # a start=True matmul wipes (pending-zeroes) its whole 2KB psum bank,
        # so zero the full 4-region tile with ONE zero-weight matmul up front
        # and accumulate every real PV matmul with start=False.
        nc.tensor.matmul(ctxT.rearrange("p j e -> p (j e)"), lhsT=zero128,
                         rhs=vo[:, 0, 0:260], start=True, stop=True,
                         skip_group_check=True)
        pend = []

        def emit_pv(p, n, pt):
            for i, kb in ((0, 2 * p), (1, 2 * p + 1)):
                for j in range(4):
                    if 128 * j + 127 < 128 * kb - q0:  # fully masked
                        continue
                    lo = i * 512 + 128 * j
                    if 128 * j < 512 - n:              # outside computed span
                        continue
                    nc.tensor.matmul(
                        ctxT[:, j, :],
                        lhsT=pt[:, lo:lo + 128],
                        rhs=vo[:, kb, 65 * h:65 * h + 65],
                        start=False, stop=(kb == 4 * qc + j),
                        skip_group_check=True)

        for p in range(npairs):
            n = 512 if p < npairs - 1 else 256
            stp = psum_st.tile([128, 1024], dt.float32, name="stp")
            for kb, lo in ((2 * p, 512 - n), (2 * p + 1, 1024 - n)):
                nc.tensor.matmul(
                    stp[:, lo:lo + n],
                    lhsT=kt[t][p0:p0 + 64, kb * 128:(kb + 1) * 128],
                    rhs=qt[t][p0:p0 + 64, q0 + 512 - n:q0 + 512],
                    start=True, stop=True)
            pt = pt_pool.tile([128, 1024], bf, name="pt")
            ptv = pt.rearrange("p (k n) -> p k n", k=2)
            stv = stp.rearrange("p (k n) -> p k n", k=2)
            nc.scalar.activation(out=ptv[:, :, 512 - n:512],
                                 in_=stv[:, :, 512 - n:512],
                                 func=F.Exp, scale=0.125)
            if p >= npairs - 2:  # diagonal pair: tri-mask both blocks
                lo = 512 - n
                nc.vector.tensor_mul(pt[:, lo:lo + 128],
                                     pt[:, lo:lo + 128], tri)
                nc.vector.tensor_mul(pt[:, 512 + lo + 128:512 + lo + 256],
                                     pt[:, 512 + lo + 128:512 + lo + 256], tri)
            pend.append((p, n, pt))
            if len(pend) > 2:
                emit_pv(*pend.pop(0))
            if fillers:
                k = (len(fillers) + npairs - 1 - p) // (npairs - p)
                for u in fillers[:k]:
                    u()
                fillers = fillers[k:]
        for pp in pend:
            emit_pv(*pp)
        for u in fillers:
            u()
        # ctxT -> sbuf, then per-q-row normalize into ctn staging
        ctf = ctf_pool.tile([128, 4, 65], dt.float32, name="ctf")
        nc.vector.tensor_copy(out=ctf, in_=ctxT)
        if dbg and qc == 0 and h == 0:
            nc.sync.dma_start(out=dbg["ctf00"].ap(), in_=ctf)
        rt = rc_pool.tile([128, 4, 1], dt.float32, name="rt")
        nc.vector.reciprocal(out=rt, in_=ctf[:, :, 64:65])
        for j in range(4):
            nc.vector.tensor_scalar_mul(
                ctn[(qc, j)][:, 64 * h:64 * h + 64],
                ctf[:, j, 0:64], rt[:, j, :])

    # ---- emission schedule ---------------------------------------------
    for m in proj_qk(wk, kt, 0, 0) + proj_qk(wq, qt, 0, 0):
        m()
    for kb in range(4):
        for m in proj_v(kb):
            m()

    def qc_fillers(qc):
        micros = []
        if qc == 0:
            for t in range(1, 4):
                micros += proj_qk(wk, kt, t, 0)
                micros += proj_qk(wq, qt, t, 0)
        if qc > 0:  # transpose last chunk's ctn into cxt (DMA xbar)
            for j in range(4):
                qb = 4 * (qc - 1) + j
                micros.append(
                    (lambda qcm=qc - 1, j=j, qb=qb: nc.sync.dma_start_transpose(
                        out=cxt[:, :, qb * 128:(qb + 1) * 128],
                        in_=ctn[(qcm, j)])))
        if qc + 1 < NQC:
            for t in range(4):
                micros += proj_qk(wk, kt, t, qc + 1)
            for kb in range(4 * qc + 4, 4 * qc + 8):
                micros += proj_v(kb)
            for t in range(4):
                micros += proj_qk(wq, qt, t, qc + 1)
        if qc == 3:  # all deferred out-projs land in the ACT-heavy tail
            for sq in range(0, 12):
                micros += out_proj(sq)
        return micros

    for qc in range(NQC):
        if qc == 1 and dbg:
            nc.sync.dma_start(out=dbg["ctn00"].ap(), in_=ctn[(0, 0)])
        for j in range(4):
            ctn[(qc, j)] = ctn_pool.tile([128, 512], bf, name="ctn")
        fillers = qc_fillers(qc)
        npairs_tot = GH * (2 * qc + 2)
        done = 0
        for h in range(GH):
            left = npairs_tot - done
            share = (len(fillers) * (2 * qc + 2) + left - 1) // left \
                if fillers else 0
            mine, fillers = fillers[:share], fillers[share:]
            done += 2 * qc + 2
            emit_attn_head(qc, h, mine)
        for u in fillers:
            u()
    for j in range(4):  # final chunk's transposes
        qb = 12 + j
        nc.sync.dma_start_transpose(out=cxt[:, :, qb * 128:(qb + 1) * 128],
                                    in_=ctn[(3, j)])
    for sq in range(12, 16):
        for m in out_proj(sq, tail=True):
            m()
    if dbg:
        nc.sync.dma_start(out=dbg["qt0"].ap(), in_=qt[0])
        nc.sync.dma_start(out=dbg["kt0"].ap(), in_=kt[0])
        nc.sync.dma_start(out=dbg["vo"].ap(), in_=vo)
        nc.sync.dma_start(out=dbg["cxt"].ap(), in_=cxt)

    return pools


def _build_nc():
    import concourse.tile as tile
    from concourse import bacc, mybir

    dt = mybir.dt
    nc = bacc.Bacc("TRN2", target_bir_lowering=False, debug=False,
                   num_devices=NCORES)
    xt_d = nc.dram_tensor("xt", [D, S], dt.bfloat16, kind="ExternalInput")
    wq_d = nc.dram_tensor("wq", [D, GD], dt.bfloat16, kind="ExternalInput")
    wk_d = nc.dram_tensor("wk", [D, GD], dt.bfloat16, kind="ExternalInput")
    wv_d = nc.dram_tensor("wv", [D, GD], dt.bfloat16, kind="ExternalInput")
    ow_d = nc.dram_tensor("ow", [GD, D], dt.bfloat16, kind="ExternalInput")
    outp = nc.dram_tensor("outp", [S, D], dt.bfloat16, kind="ExternalOutput")
    import os as _os
    dbg = None
    if _os.environ.get("KDEBUG"):
        dbg = {
            "qt0": nc.dram_tensor("dqt0", [128, S], dt.bfloat16, kind="ExternalOutput"),
            "kt0": nc.dram_tensor("dkt0", [128, S], dt.bfloat16, kind="ExternalOutput"),
            "vo": nc.dram_tensor("dvo", [128, NKB, GH * 65], dt.bfloat16, kind="ExternalOutput"),
            "cxt": nc.dram_tensor("dcxt", [128, 4, S], dt.bfloat16, kind="ExternalOutput"),
            "ctn00": nc.dram_tensor("dctn00", [128, 512], dt.bfloat16, kind="ExternalOutput"),
            "ctf00": nc.dram_tensor("dctf00", [128, 4, 65], dt.float32, kind="ExternalOutput"),
        }

    with tile.TileContext(nc) as tc:
        pools = _build_body(tc, nc, mybir, xt_d, wq_d, wk_d, wv_d, ow_d, outp, dbg)
        pools.close()
    nc.compile()
    return nc


LAST_RESULTS = None


def kernel(batch, w_query, w_key, w_value, out_w, out_b):
    global LAST_RESULTS
    import os
    from concourse import bass_utils

    try:  # BASS_TRACE needs the axon NTFF hook; without it the run crashes
        from antenv.axon_hooks import get_axon_ntff_profile_hook  # noqa: F401
    except ImportError:
        os.environ.setdefault("BASS_NEVER_TRACE", "1")

    batch = np.asarray(batch, dtype=np.float32)
    w_query = np.asarray(w_query, dtype=np.float32)
    w_key = np.asarray(w_key, dtype=np.float32)
    w_value = np.asarray(w_value, dtype=np.float32)
    out_w = np.asarray(out_w, dtype=np.float32)
    out_b = np.asarray(out_b, dtype=np.float32)

    if "nc" not in _cache:
        _cache["nc"] = _build_nc()
    nc = _cache["nc"]

    xts = [np.ascontiguousarray(batch[b].T).astype(BF16) for b in range(B)]
    slc = [slice(g * GD, (g + 1) * GD) for g in range(2)]
    wqs = [np.ascontiguousarray(w_query[:, s]).astype(BF16) for s in slc]
    wks = [np.ascontiguousarray(w_key[:, s]).astype(BF16) for s in slc]
    wvs = [np.ascontiguousarray(w_value[:, s]).astype(BF16) for s in slc]
    ows = [np.ascontiguousarray(out_w[s, :]).astype(BF16) for s in slc]
    in_maps = []
    for c in range(NCORES):
        b, g = divmod(c, 2)
        in_maps.append({
            "xt": xts[b], "wq": wqs[g], "wk": wks[g],
            "wv": wvs[g], "ow": ows[g],
        })

    res = bass_utils.run_bass_kernel_spmd(
        nc, in_maps, core_ids=list(range(NCORES)),
    )
    LAST_RESULTS = res

    out = np.empty((B, S, D), np.float32)
    for b in range(B):
        out[b] = res.results[2 * b]["outp"].astype(np.float32) \
            + res.results[2 * b + 1]["outp"].astype(np.float32) \
            + out_b[None, :]
    return out
